# revision 4
# baseline (speedup 1.0000x reference)
"""CapsuleLayer kernel for 8x TRN2 NeuronCores (Bass/Tile, SPMD).

Math (reference collapses because routing logits b stay zero):
  s[b,o,h,w]  = sum_ic conv2d(u[b,ic], W[ic], SAME) + sum_ic bias[ic]
              = conv2d(u[b] as 64ch, Wcat[256,64,5,5]) + bias_sum      (one conv)
  r(h,w)      = 1 / (8 * nvalid(h,w))              (input-independent constant)
  p           = r * s ; sq[oc] = sum_od p^2
  v           = p * sq/((1+sq)*sqrt(sq+1e-9))
  out[b,oc,od,h,w] = v

Sharding: 8 cores = (batch b in 0..4) x (OC half in 0..2). Each core runs a
64->128-channel 5x5 conv over one 128x128 image + squash, fully on-chip.

Conv as matmuls: 15 accumulating PE matmuls per 512-pixel tile with K=128 =
2 kernel-rows x 64 in-channels (SBUF holds the padded image twice, the second
copy shifted one row, so paired kernel rows share one matmul). float32r
dtype: full PE speed (1 cyc/col at N=512) at ~1e-4 accuracy.

Squash: per-pixel cross-partition reduce/broadcast via PE matmuls with 0/1
masks (accumulating masked reduces build a [32,512] per-group tensor), the
scalar chain uses only {square, ln, exp} (one ACT table set):
  G = r * exp(0.5*ln(sq+1e-9) - ln(1+sq));  v = s * G_broadcast
"""

import numpy as np


def _ensure_path():
    try:
        import concourse.bass  # noqa: F401
    except ImportError:
        import sys

        for p in ("/opt/trn_rl_repo", "/root/.axon_site/_ro/trn_rl_repo"):
            if p not in sys.path:
                sys.path.insert(0, p)
        import concourse.bass  # noqa: F401


B, IC, CIN, H, W = 4, 4, 16, 128, 128
KS, OC, OD = 5, 8, 32
CC = IC * CIN            # 64 contraction channels
NOCH = 128               # out channels per core (4 capsules x 32 dims)
PADH, PADW = H + 5, W + 4   # 133 x 132 (extra pad row for the shifted copy)
NPIX = H * W
TPX = 512                # pixels per tile (4 rows)
NT = NPIX // TPX         # 32 tiles
GT = 8                   # tiles per group
NG = NT // GT            # 4 groups
NKT = 15                 # matmuls per conv tile: 3 row-pairs x 5 kw

_BUILD_CACHE = {}


def _build_program():
    """Build the SPMD Bass program (same for every core)."""
    if "nc" in _BUILD_CACHE:
        return _BUILD_CACHE["nc"]
    _ensure_path()
    import concourse.bacc as bacc
    import concourse.mybir as mybir
    import concourse.tile as tile

    f32 = mybir.dt.float32
    f32r = mybir.dt.float32r
    AF = mybir.ActivationFunctionType
    OP = mybir.AluOpType

    nc = bacc.Bacc("TRN2", target_bir_lowering=False, debug=False, num_devices=8)

    upad_d = nc.dram_tensor("upad", [128, PADH * PADW], f32r, kind="ExternalInput").ap()
    wt_d = nc.dram_tensor("wt", [128, NKT * NOCH], f32r, kind="ExternalInput").ap()
    bias_d = nc.dram_tensor("bias", [128, 1], f32, kind="ExternalInput").ap()
    mred_d = nc.dram_tensor("mred", [128, GT * 32], f32r, kind="ExternalInput").ap()
    sel_d = nc.dram_tensor("sel", [32, GT * NOCH], f32r, kind="ExternalInput").ap()
    rr_d = nc.dram_tensor("rr", [32, 2 * NG * TPX], f32, kind="ExternalInput").ap()
    out_d = nc.dram_tensor("out", [128, NPIX], f32, kind="ExternalOutput").ap()

    with tile.TileContext(nc) as tc:
        with (
            tc.tile_pool(name="const", bufs=1) as cpool,
            tc.tile_pool(name="sg", bufs=2) as sgpool,
            tc.tile_pool(name="sq", bufs=3) as sqpool,
            tc.tile_pool(name="chain", bufs=2) as chpool,
            tc.tile_pool(name="gv", bufs=2) as gvpool,
            tc.tile_pool(name="vout", bufs=3) as vpool,
            tc.tile_pool(name="cps", bufs=3, space="PSUM") as cps,
            tc.tile_pool(name="gps", bufs=2, space="PSUM") as gps,
            tc.tile_pool(name="bps", bufs=2, space="PSUM") as bps,
        ):
            wt_t = cpool.tile([128, NKT * NOCH], f32r)
            nc.sync.dma_start(wt_t[:], wt_d[:])
            bias_t = cpool.tile([128, 1], f32)
            nc.sync.dma_start(bias_t[:], bias_d[:])
            mred_t = cpool.tile([128, GT * 32], f32r)
            nc.sync.dma_start(mred_t[:], mred_d[:])
            sel_t = cpool.tile([32, GT * NOCH], f32r)
            nc.sync.dma_start(sel_t[:], sel_d[:])
            rr_t = cpool.tile([32, 2 * NG * TPX], f32)
            nc.sync.dma_start(rr_t[:], rr_d[:])
            eps_t = cpool.tile([128, 1], f32)
            nc.vector.memset(eps_t[:], 1e-9)
            upad_t = cpool.tile([128, PADH * PADW], f32r)
            # split the big (9 MB) image load so early conv tiles can start
            # before the whole padded image lands
            upad3 = upad_t[:].rearrange("p (y x) -> p y x", x=PADW)
            usrc3 = upad_d.rearrange("p (y x) -> p y x", x=PADW)
            row_chunks = [(0, 38), (38, 70), (70, 102), (102, PADH)]
            for r0, r1 in row_chunks:
                nc.sync.dma_start(upad3[:, r0:r1, :], usrc3[:, r0:r1, :])

            s_tiles = {}
            g_tiles = {}

            def emit_conv_tile(g, j, gp_ref):
                t = g * GT + j
                y0 = 4 * t
                cp = cps.tile([128, TPX], f32, tag="convps")
                ti = 0
                for dyp in range(3):
                    for kw in range(KS):
                        rhs = upad3[:, y0 + 2 * dyp : y0 + 2 * dyp + 4, kw : kw + W]
                        nc.tensor.matmul(
                            cp[:],
                            wt_t[:, ti * NOCH : (ti + 1) * NOCH],
                            rhs,
                            start=(ti == 0),
                            stop=(ti == NKT - 1),
                        )
                        ti += 1
                s_sl = s_tiles[g][:, j * TPX : (j + 1) * TPX]
                nc.vector.tensor_scalar(
                    s_sl, cp[:], bias_t[:, 0:1], None, OP.add
                )
                sq = sqpool.tile([128, TPX], f32r, tag="sqt")
                nc.scalar.activation(sq[:], s_sl, AF.Square)
                nc.tensor.matmul(
                    gp_ref[:],
                    mred_t[:, j * 32 : (j + 1) * 32],
                    sq[:],
                    start=(j == 0),
                    stop=(j == GT - 1),
                )

            def emit_chain(g, gp_ref):
                r4 = rr_t[:, g * TPX : (g + 1) * TPX]
                r1 = rr_t[:, (NG + g) * TPX : (NG + g + 1) * TPX]
                sqv = chpool.tile([32, TPX], f32, tag="sqv")
                nc.vector.tensor_mul(sqv[:], gp_ref[:], r4)
                ln_a = chpool.tile([32, TPX], f32, tag="ln_a")
                nc.scalar.activation(ln_a[:], sqv[:], AF.Ln, bias=eps_t[0:32, 0:1])
                ln_b = chpool.tile([32, TPX], f32, tag="ln_b")
                nc.scalar.activation(ln_b[:], sqv[:], AF.Ln, bias=1.0)
                dd = chpool.tile([32, TPX], f32, tag="dd")
                nc.vector.scalar_tensor_tensor(
                    dd[:], ln_a[:], 0.5, ln_b[:], OP.mult, OP.subtract
                )
                ee = chpool.tile([32, TPX], f32, tag="ee")
                nc.scalar.activation(ee[:], dd[:], AF.Exp)
                gt_ = gvpool.tile([32, TPX], f32r, tag="g32")
                nc.vector.tensor_mul(gt_[:], ee[:], r1)
                g_tiles[g] = gt_

            def emit_phase2(g, j):
                t = g * GT + j
                gb = bps.tile([128, TPX], f32, tag="bcast")
                nc.tensor.matmul(
                    gb[:],
                    sel_t[:, j * NOCH : (j + 1) * NOCH],
                    g_tiles[g][:],
                    start=True,
                    stop=True,
                )
                v = vpool.tile([128, TPX], f32, tag="vout")
                s_sl = s_tiles[g][:, j * TPX : (j + 1) * TPX]
                nc.vector.tensor_mul(v[:], s_sl, gb[:])
                nc.sync.dma_start(out_d[:, t * TPX : (t + 1) * TPX], v[:])

            for g in range(NG):
                s_tiles[g] = sgpool.tile(
                    [128, GT * TPX], f32, tag="sgroup", name=f"sgroup{g}"
                )
                gp = gps.tile([32, TPX], f32, tag="redps")
                for j in range(GT):
                    emit_conv_tile(g, j, gp)
                    if g > 0 and j >= 1:
                        emit_phase2(g - 1, j - 1)
                emit_chain(g, gp)
                if g > 0:
                    emit_phase2(g - 1, GT - 1)
            for j in range(GT):
                emit_phase2(NG - 1, j)

    nc.compile()
    _BUILD_CACHE["nc"] = nc
    return nc


def _host_prep(u, Wf, bias):
    """Per-core input arrays. u [4,4,16,128,128], Wf [4,256,16,5,5], bias [4,256]."""
    u = np.ascontiguousarray(u, dtype=np.float32)
    Wf = np.ascontiguousarray(Wf, dtype=np.float32)
    bias = np.ascontiguousarray(bias, dtype=np.float32)

    # r(h,w) = 1/(8*nvalid); nvalid = clipped 5x5 window size
    nv = np.minimum(np.arange(H) + 2, H - 1) - np.maximum(np.arange(H) - 2, 0) + 1
    nvalid = np.outer(nv, nv).astype(np.float64)
    r = (1.0 / (8.0 * nvalid)).astype(np.float32)          # [H, W]

    # RR[p, g*TPX + n] layout: row p -> (j = p//4), value depends on pixel only
    rr = np.empty((32, 2 * NG * TPX), np.float32)
    rflat = r.reshape(H * W)
    for g in range(NG):
        for p in range(32):
            j = p // 4
            t = g * GT + j
            px = rflat[t * TPX : (t + 1) * TPX]
            rr[p, g * TPX : (g + 1) * TPX] = px * px
            rr[p, (NG + g) * TPX : (NG + g + 1) * TPX] = px

    # reduce masks: MRED[p, j*32+m] = 1 if m == 4*j + p//32
    mred = np.zeros((128, GT * 32), np.float32)
    for j in range(GT):
        for p in range(128):
            mred[p, j * 32 + 4 * j + p // 32] = 1.0
    # broadcast sel: SEL[p, j*128+m] = 1 if p == 4*j + m//32
    sel = np.zeros((32, GT * NOCH), np.float32)
    for j in range(GT):
        for m in range(NOCH):
            sel[4 * j + m // 32, j * NOCH + m] = 1.0

    bias_sum = bias.sum(axis=0)                            # [256]

    # weights: WT[p, ti*128 + o]; ti = dyp*5+kw; p = hlf64*64 + ic*16 + cid
    # lhsT[p, o] = W[ic, half*128+o, cid, 2*dyp+hlf64, kw] (0 for dy==5)
    wts = []
    for half in range(2):
        wt = np.zeros((128, NKT * NOCH), np.float32)
        Wh = Wf[:, half * NOCH : (half + 1) * NOCH]        # [4, 128, 16, 5, 5]
        for dyp in range(3):
            for kw in range(KS):
                ti = dyp * 5 + kw
                for h64 in range(2):
                    dy = 2 * dyp + h64
                    if dy >= KS:
                        continue
                    # [4,128,16] -> [4,16,128] -> [64,128]
                    blk = Wh[:, :, :, dy, kw].transpose(0, 2, 1).reshape(64, NOCH)
                    wt[h64 * 64 : (h64 + 1) * 64, ti * NOCH : (ti + 1) * NOCH] = blk
        wts.append(wt)

    # padded image per batch: pad[c, yy, xx]; second copy shifted +1 row
    upads = []
    for b in range(B):
        pad = np.zeros((CC, PADH, PADW), np.float32)
        pad[:, 2 : 2 + H, 2 : 2 + W] = u[b].reshape(CC, H, W)
        up = np.empty((128, PADH * PADW), np.float32)
        up[0:64] = pad.reshape(CC, -1)
        sh = np.zeros_like(pad)
        sh[:, 0 : PADH - 1] = pad[:, 1:PADH]
        up[64:128] = sh.reshape(CC, -1)
        upads.append(up)

    in_maps = []
    for c in range(8):
        b, half = c // 2, c % 2
        in_maps.append(
            {
                "upad": upads[b],
                "wt": wts[half],
                "bias": bias_sum[half * NOCH : (half + 1) * NOCH]
                .reshape(128, 1)
                .copy(),
                "mred": mred,
                "sel": sel,
                "rr": rr,
            }
        )
    return in_maps


def _gather(results):
    out = np.empty((B, OC, OD, H, W), np.float32)
    for c in range(8):
        b, half = c // 2, c % 2
        o = results[c]["out"]                              # [128, NPIX]
        out[b, half * 4 : (half + 1) * 4] = o.reshape(4, OD, H, W)
    return out


def run(u, W, bias, trace=False):
    _ensure_path()
    from concourse.bass_utils import run_bass_kernel_spmd

    nc = _build_program()
    in_maps = _host_prep(u, W, bias)
    res = run_bass_kernel_spmd(nc, in_maps, list(range(8)), trace=trace)
    return _gather(res.results), res


def kernel(u, W, bias):
    out, _ = run(u, W, bias, trace=False)
    return out


# revision 12
# speedup vs baseline: 1.1327x; 1.1327x over previous
"""CapsuleLayer kernel for 8x TRN2 NeuronCores (Bass/Tile, SPMD).

Math (reference collapses because routing logits b stay zero):
  s[b,o,h,w]  = sum_ic conv2d(u[b,ic], W[ic], SAME) + sum_ic bias[ic]
              = conv2d(u[b] as 64ch, Wcat[256,64,5,5]) + bias_sum      (one conv)
  r(h,w)      = 1 / (8 * nvalid(h,w))              (input-independent constant)
  p           = r * s ; sq[oc] = sum_od p^2
  v           = p * sq/((1+sq)*sqrt(sq+1e-9))
  out[b,oc,od,h,w] = v

Sharding: 8 cores = (batch b in 0..4) x (OC half in 0..2). Each core runs a
64->128-channel 5x5 conv over one 128x128 image + squash, fully on-chip.

Conv as matmuls: 15 accumulating PE matmuls per 512-pixel tile with K=128 =
2 kernel-rows x 64 in-channels (SBUF holds the padded image twice, the second
copy shifted one row, so paired kernel rows share one matmul). float32r
dtype: full PE speed (1 cyc/col at N=512) at ~1e-4 accuracy.

Squash: per-pixel cross-partition reduce/broadcast via PE matmuls with 0/1
masks (accumulating masked reduces build a [32,512] per-group tensor), the
scalar chain uses only {square, ln, exp} (one ACT table set):
  G = r * exp(0.5*ln(sq+1e-9) - ln(1+sq));  v = s * G_broadcast
"""

import numpy as np


def _ensure_path():
    try:
        import concourse.bass  # noqa: F401
    except ImportError:
        import sys

        for p in ("/opt/trn_rl_repo", "/root/.axon_site/_ro/trn_rl_repo"):
            if p not in sys.path:
                sys.path.insert(0, p)
        import concourse.bass  # noqa: F401


B, IC, CIN, H, W = 4, 4, 16, 128, 128
KS, OC, OD = 5, 8, 32
CC = IC * CIN            # 64 contraction channels
NOCH = 128               # out channels per core (4 capsules x 32 dims)
PADH, PADW = H + 5, W + 4   # 133 x 132 (extra pad row for the shifted copy)
NPIX = H * W
TPX = 512                # pixels per tile (4 rows)
NT = NPIX // TPX         # 32 tiles
GT = 8                   # tiles per group
NG = NT // GT            # 4 groups
NKT = 15                 # matmuls per conv tile: 3 row-pairs x 5 kw

_BUILD_CACHE = {}


def _build_program():
    """Build the SPMD Bass program (same for every core)."""
    if "nc" in _BUILD_CACHE:
        return _BUILD_CACHE["nc"]
    _ensure_path()
    import concourse.bacc as bacc
    import concourse.mybir as mybir
    import concourse.tile as tile

    f32 = mybir.dt.float32
    f16 = mybir.dt.float16
    AF = mybir.ActivationFunctionType
    OP = mybir.AluOpType

    # Square/Ln/Exp/Identity all live in the 'natural_log_exp_and_others' ACT
    # table set, but the default set picker uses a different home set per
    # function (2 table reloads x 1.3us per group). Restrict the choice so a
    # single table load covers the whole kernel.
    if not getattr(bacc, "_capsule_act_patch", False):
        _orig_tables = bacc.get_activation_tables

        def _one_set_tables(arch):
            t = _orig_tables(arch)
            keep = "natural_log_exp_and_others"
            if keep in t:
                t = {k: (v if k == keep else set()) for k, v in t.items()}
            return t

        bacc.get_activation_tables = _one_set_tables
        bacc._capsule_act_patch = True

    nc = bacc.Bacc("TRN2", target_bir_lowering=False, debug=False, num_devices=8)

    upad_d = nc.dram_tensor("upad", [128, PADH * PADW], f16, kind="ExternalInput").ap()
    wt_d = nc.dram_tensor("wt", [128, NKT * NOCH], f16, kind="ExternalInput").ap()
    bias_d = nc.dram_tensor("bias", [128, 1], f32, kind="ExternalInput").ap()
    mred_d = nc.dram_tensor("mred", [128, GT * 32], f16, kind="ExternalInput").ap()
    sel_d = nc.dram_tensor("sel", [32, GT * NOCH], f16, kind="ExternalInput").ap()
    rr_d = nc.dram_tensor("rr", [32, 2 * NG * TPX], f32, kind="ExternalInput").ap()
    out_d = nc.dram_tensor("out", [128, NPIX], f32, kind="ExternalOutput").ap()

    with tile.TileContext(nc) as tc:
        with (
            tc.tile_pool(name="const", bufs=1) as cpool,
            tc.tile_pool(name="sg", bufs=2) as sgpool,
            tc.tile_pool(name="sq", bufs=3) as sqpool,
            tc.tile_pool(name="chain", bufs=2) as chpool,
            tc.tile_pool(name="gv", bufs=2) as gvpool,
            tc.tile_pool(name="vout", bufs=3) as vpool,
            tc.tile_pool(name="cps", bufs=3, space="PSUM") as cps,
            tc.tile_pool(name="gps", bufs=2, space="PSUM") as gps,
            tc.tile_pool(name="bps", bufs=2, space="PSUM") as bps,
        ):
            # DMA order matters: the first conv tiles need wt + the first image
            # rows; everything else can land while the PE is already running.
            wt_t = cpool.tile([128, NKT * NOCH], f16)
            nc.sync.dma_start(wt_t[:], wt_d[:])
            upad_t = cpool.tile([128, PADH * PADW], f16)
            upad3 = upad_t[:].rearrange("p (y x) -> p y x", x=PADW)
            usrc3 = upad_d.rearrange("p (y x) -> p y x", x=PADW)
            row_chunks = [(0, 24), (24, 60), (60, 96), (96, PADH)]
            r0, r1 = row_chunks[0]
            nc.sync.dma_start(upad3[:, r0:r1, :], usrc3[:, r0:r1, :])
            bias_t = cpool.tile([128, 1], f32)
            nc.sync.dma_start(bias_t[:], bias_d[:])
            mred_t = cpool.tile([128, GT * 32], f16)
            nc.sync.dma_start(mred_t[:], mred_d[:])
            r0, r1 = row_chunks[1]
            nc.sync.dma_start(upad3[:, r0:r1, :], usrc3[:, r0:r1, :])
            sel_t = cpool.tile([32, GT * NOCH], f16)
            nc.sync.dma_start(sel_t[:], sel_d[:])
            rr_t = cpool.tile([32, 2 * NG * TPX], f32)
            nc.sync.dma_start(rr_t[:], rr_d[:])
            eps_t = cpool.tile([128, 1], f32)
            nc.vector.memset(eps_t[:], 1e-9)
            for r0, r1 in row_chunks[2:]:
                nc.sync.dma_start(upad3[:, r0:r1, :], usrc3[:, r0:r1, :])

            s_tiles = {}
            g_tiles = {}

            def emit_conv_tile(g, j, gp_ref):
                t = g * GT + j
                y0 = 4 * t
                cp = cps.tile([128, TPX], f32, tag="convps")
                ti = 0
                for dyp in range(3):
                    for kw in range(KS):
                        rhs = upad3[:, y0 + 2 * dyp : y0 + 2 * dyp + 4, kw : kw + W]
                        nc.tensor.matmul(
                            cp[:],
                            wt_t[:, ti * NOCH : (ti + 1) * NOCH],
                            rhs,
                            start=(ti == 0),
                            stop=(ti == NKT - 1),
                        )
                        ti += 1
                s_sl = s_tiles[g][:, j * TPX : (j + 1) * TPX]
                nc.scalar.add(s_sl, cp[:], bias_t[:, 0:1])
                sq = sqpool.tile([128, TPX], f16, tag="sqt")
                nc.scalar.activation(sq[:], s_sl, AF.Square)
                nc.tensor.matmul(
                    gp_ref[:],
                    mred_t[:, j * 32 : (j + 1) * 32],
                    sq[:],
                    start=(j == 0),
                    stop=(j == GT - 1),
                )

            def emit_chain(g, gp_ref):
                r4 = rr_t[:, g * TPX : (g + 1) * TPX]
                r1 = rr_t[:, (NG + g) * TPX : (NG + g + 1) * TPX]
                sqv = chpool.tile([32, TPX], f32, tag="sqv")
                nc.vector.tensor_mul(sqv[:], gp_ref[:], r4)
                ln_a = chpool.tile([32, TPX], f32, tag="ln_a")
                nc.scalar.activation(ln_a[:], sqv[:], AF.Ln, bias=eps_t[0:32, 0:1])
                ln_b = chpool.tile([32, TPX], f32, tag="ln_b")
                nc.scalar.activation(ln_b[:], sqv[:], AF.Ln, bias=1.0)
                dd = chpool.tile([32, TPX], f32, tag="dd")
                nc.vector.scalar_tensor_tensor(
                    dd[:], ln_a[:], 0.5, ln_b[:], OP.mult, OP.subtract
                )
                ee = chpool.tile([32, TPX], f32, tag="ee")
                nc.scalar.activation(ee[:], dd[:], AF.Exp)
                gt_ = gvpool.tile([32, TPX], f16, tag="g32")
                nc.vector.tensor_mul(gt_[:], ee[:], r1)
                g_tiles[g] = gt_

            def emit_phase2(g, j):
                t = g * GT + j
                gb = bps.tile([128, TPX], f32, tag="bcast")
                nc.tensor.matmul(
                    gb[:],
                    sel_t[:, j * NOCH : (j + 1) * NOCH],
                    g_tiles[g][:],
                    start=True,
                    stop=True,
                )
                v = vpool.tile([128, TPX], f32, tag="vout")
                s_sl = s_tiles[g][:, j * TPX : (j + 1) * TPX]
                nc.vector.tensor_mul(v[:], s_sl, gb[:])
                nc.sync.dma_start(out_d[:, t * TPX : (t + 1) * TPX], v[:])

            for g in range(NG):
                s_tiles[g] = sgpool.tile(
                    [128, GT * TPX], f32, tag="sgroup", name=f"sgroup{g}"
                )
                gp = gps.tile([32, TPX], f32, tag="redps")
                for j in range(GT):
                    emit_conv_tile(g, j, gp)
                    if g > 0 and j >= 1:
                        emit_phase2(g - 1, j - 1)
                emit_chain(g, gp)
                if g > 0:
                    emit_phase2(g - 1, GT - 1)
            for j in range(GT):
                emit_phase2(NG - 1, j)

    nc.compile()
    _BUILD_CACHE["nc"] = nc
    return nc


def _host_prep(u, Wf, bias):
    """Per-core input arrays. u [4,4,16,128,128], Wf [4,256,16,5,5], bias [4,256]."""
    u = np.ascontiguousarray(u, dtype=np.float32)
    Wf = np.ascontiguousarray(Wf, dtype=np.float32)
    bias = np.ascontiguousarray(bias, dtype=np.float32)

    # r(h,w) = 1/(8*nvalid); nvalid = clipped 5x5 window size
    nv = np.minimum(np.arange(H) + 2, H - 1) - np.maximum(np.arange(H) - 2, 0) + 1
    nvalid = np.outer(nv, nv).astype(np.float64)
    r = (1.0 / (8.0 * nvalid)).astype(np.float32)          # [H, W]

    # RR[p, g*TPX + n] layout: row p -> (j = p//4), value depends on pixel only
    rr = np.empty((32, 2 * NG * TPX), np.float32)
    rflat = r.reshape(H * W)
    for g in range(NG):
        for p in range(32):
            j = p // 4
            t = g * GT + j
            px = rflat[t * TPX : (t + 1) * TPX]
            rr[p, g * TPX : (g + 1) * TPX] = px * px
            rr[p, (NG + g) * TPX : (NG + g + 1) * TPX] = px

    # reduce masks: MRED[p, j*32+m] = 1 if m == 4*j + p//32
    mred = np.zeros((128, GT * 32), np.float16)
    for j in range(GT):
        for p in range(128):
            mred[p, j * 32 + 4 * j + p // 32] = 1.0
    # broadcast sel: SEL[p, j*128+m] = 1 if p == 4*j + m//32
    sel = np.zeros((32, GT * NOCH), np.float16)
    for j in range(GT):
        for m in range(NOCH):
            sel[4 * j + m // 32, j * NOCH + m] = 1.0

    bias_sum = bias.sum(axis=0)                            # [256]

    # weights: WT[p, ti*128 + o]; ti = dyp*5+kw; p = hlf64*64 + ic*16 + cid
    # lhsT[p, o] = W[ic, half*128+o, cid, 2*dyp+hlf64, kw] (0 for dy==5)
    wts = []
    for half in range(2):
        wt = np.zeros((128, NKT * NOCH), np.float16)
        Wh = Wf[:, half * NOCH : (half + 1) * NOCH]        # [4, 128, 16, 5, 5]
        for dyp in range(3):
            for kw in range(KS):
                ti = dyp * 5 + kw
                for h64 in range(2):
                    dy = 2 * dyp + h64
                    if dy >= KS:
                        continue
                    # [4,128,16] -> [4,16,128] -> [64,128]
                    blk = Wh[:, :, :, dy, kw].transpose(0, 2, 1).reshape(64, NOCH)
                    wt[h64 * 64 : (h64 + 1) * 64, ti * NOCH : (ti + 1) * NOCH] = blk
        wts.append(wt)

    # padded image per batch: pad[c, yy, xx]; second copy shifted +1 row
    upads = []
    for b in range(B):
        pad = np.zeros((CC, PADH, PADW), np.float16)
        pad[:, 2 : 2 + H, 2 : 2 + W] = u[b].reshape(CC, H, W)
        up = np.empty((128, PADH * PADW), np.float16)
        up[0:64] = pad.reshape(CC, -1)
        sh = np.zeros_like(pad)
        sh[:, 0 : PADH - 1] = pad[:, 1:PADH]
        up[64:128] = sh.reshape(CC, -1)
        upads.append(up)

    in_maps = []
    for c in range(8):
        b, half = c // 2, c % 2
        in_maps.append(
            {
                "upad": upads[b],
                "wt": wts[half],
                "bias": bias_sum[half * NOCH : (half + 1) * NOCH]
                .reshape(128, 1)
                .copy(),
                "mred": mred,
                "sel": sel,
                "rr": rr,
            }
        )
    return in_maps


def _gather(results):
    out = np.empty((B, OC, OD, H, W), np.float32)
    for c in range(8):
        b, half = c // 2, c % 2
        o = results[c]["out"]                              # [128, NPIX]
        out[b, half * 4 : (half + 1) * 4] = o.reshape(4, OD, H, W)
    return out


def run(u, W, bias, trace=False):
    _ensure_path()
    from concourse.bass_utils import run_bass_kernel_spmd

    nc = _build_program()
    in_maps = _host_prep(u, W, bias)
    res = run_bass_kernel_spmd(nc, in_maps, list(range(8)), trace=trace)
    return _gather(res.results), res


def kernel(u, W, bias):
    out, _ = run(u, W, bias, trace=False)
    return out


# revision 16
# speedup vs baseline: 1.1632x; 1.0269x over previous
"""CapsuleLayer kernel for 8x TRN2 NeuronCores (Bass/Tile, SPMD).

Math (reference collapses because routing logits b stay zero):
  s[b,o,h,w]  = sum_ic conv2d(u[b,ic], W[ic], SAME) + sum_ic bias[ic]
              = conv2d(u[b] as 64ch, Wcat[256,64,5,5]) + bias_sum      (one conv)
  r(h,w)      = 1 / (8 * nvalid(h,w))              (input-independent constant)
  p           = r * s ; sq[oc] = sum_od p^2
  v           = p * sq/((1+sq)*sqrt(sq+1e-9))
  out[b,oc,od,h,w] = v

Sharding: 8 cores = (batch b in 0..4) x (OC half in 0..2). Each core runs a
64->128-channel 5x5 conv over one 128x128 image + squash, fully on-chip.

Conv as matmuls: 15 accumulating PE matmuls per 512-pixel tile with K=128 =
2 kernel-rows x 64 in-channels (SBUF holds the padded image twice, the second
copy shifted one row, so paired kernel rows share one matmul). float32r
dtype: full PE speed (1 cyc/col at N=512) at ~1e-4 accuracy.

Squash: per-pixel cross-partition reduce/broadcast via PE matmuls with 0/1
masks (accumulating masked reduces build a [32,512] per-group tensor), the
scalar chain uses only {square, ln, exp} (one ACT table set):
  G = r * exp(0.5*ln(sq+1e-9) - ln(1+sq));  v = s * G_broadcast
"""

import numpy as np


def _ensure_path():
    try:
        import concourse.bass  # noqa: F401
    except ImportError:
        import sys

        for p in ("/opt/trn_rl_repo", "/root/.axon_site/_ro/trn_rl_repo"):
            if p not in sys.path:
                sys.path.insert(0, p)
        import concourse.bass  # noqa: F401


B, IC, CIN, H, W = 4, 4, 16, 128, 128
KS, OC, OD = 5, 8, 32
CC = IC * CIN            # 64 contraction channels
NOCH = 128               # out channels per core (4 capsules x 32 dims)
PADH, PADW = H + 5, W + 4   # 133 x 132 (extra pad row for the shifted copy)
NPIX = H * W
TPX = 512                # pixels per tile (4 rows)
NT = NPIX // TPX         # 32 tiles
GT = 8                   # max tiles per group (mask/sel layouts sized for this)
GROUPS = [8, 8, 8, 4, 4]  # tiles per squash group (small tail groups)
NGR = len(GROUPS)
NKT = 15                 # matmuls per conv tile: 3 row-pairs x 5 kw

_BUILD_CACHE = {}


def _build_program():
    """Build the SPMD Bass program (same for every core)."""
    if "nc" in _BUILD_CACHE:
        return _BUILD_CACHE["nc"]
    _ensure_path()
    import concourse.bacc as bacc
    import concourse.mybir as mybir
    import concourse.tile as tile

    f32 = mybir.dt.float32
    f16 = mybir.dt.float16
    AF = mybir.ActivationFunctionType
    OP = mybir.AluOpType

    # Square/Ln/Exp/Identity all live in the 'natural_log_exp_and_others' ACT
    # table set, but the default set picker uses a different home set per
    # function (2 table reloads x 1.3us per group). Restrict the choice so a
    # single table load covers the whole kernel.
    if not getattr(bacc, "_capsule_act_patch", False):
        _orig_tables = bacc.get_activation_tables

        def _one_set_tables(arch):
            t = _orig_tables(arch)
            keep = "natural_log_exp_and_others"
            if keep in t:
                t = {k: (v if k == keep else set()) for k, v in t.items()}
            return t

        bacc.get_activation_tables = _one_set_tables
        bacc._capsule_act_patch = True

    nc = bacc.Bacc("TRN2", target_bir_lowering=False, debug=False, num_devices=8)

    upad_d = nc.dram_tensor("upad", [128, PADH * PADW], f16, kind="ExternalInput").ap()
    wt_d = nc.dram_tensor("wt", [128, NKT * NOCH], f16, kind="ExternalInput").ap()
    bias_d = nc.dram_tensor("bias", [128, 1], f32, kind="ExternalInput").ap()
    mred_d = nc.dram_tensor("mred", [128, GT * 32], f16, kind="ExternalInput").ap()
    sel_d = nc.dram_tensor("sel", [32, GT * NOCH], f16, kind="ExternalInput").ap()
    rr_d = nc.dram_tensor("rr", [32, 2 * NGR * TPX], f32, kind="ExternalInput").ap()
    out_d = nc.dram_tensor("out", [128, NPIX], f32, kind="ExternalOutput").ap()

    with tile.TileContext(nc) as tc:
        with (
            tc.tile_pool(name="const", bufs=1) as cpool,
            tc.tile_pool(name="sg", bufs=2) as sgpool,
            tc.tile_pool(name="sq", bufs=3) as sqpool,
            tc.tile_pool(name="chain", bufs=2) as chpool,
            tc.tile_pool(name="gv", bufs=2) as gvpool,
            tc.tile_pool(name="vout", bufs=3) as vpool,
            tc.tile_pool(name="cps", bufs=3, space="PSUM") as cps,
            tc.tile_pool(name="gps", bufs=2, space="PSUM") as gps,
            tc.tile_pool(name="bps", bufs=2, space="PSUM") as bps,
        ):
            # DMA order matters: the first conv tiles need wt + the first image
            # rows. wt goes on the Sync HWDGE ring, the image chunks on the
            # Scalar HWDGE ring so the two streams land in parallel.
            wt_t = cpool.tile([128, NKT * NOCH], f16)
            nc.sync.dma_start(wt_t[:], wt_d[:])
            upad_t = cpool.tile([128, PADH * PADW], f16)
            upad3 = upad_t[:].rearrange("p (y x) -> p y x", x=PADW)
            usrc3 = upad_d.rearrange("p (y x) -> p y x", x=PADW)
            row_chunks = [(0, 14), (14, 44), (44, 74), (74, 104), (104, PADH)]
            for r0, r1 in row_chunks:
                nc.scalar.dma_start(upad3[:, r0:r1, :], usrc3[:, r0:r1, :])
            bias_t = cpool.tile([128, 1], f32)
            nc.sync.dma_start(bias_t[:], bias_d[:])
            mred_t = cpool.tile([128, GT * 32], f16)
            nc.sync.dma_start(mred_t[:], mred_d[:])
            sel_t = cpool.tile([32, GT * NOCH], f16)
            nc.sync.dma_start(sel_t[:], sel_d[:])
            rr_t = cpool.tile([32, 2 * NGR * TPX], f32)
            nc.sync.dma_start(rr_t[:], rr_d[:])
            eps_t = cpool.tile([128, 1], f32)
            nc.vector.memset(eps_t[:], 1e-9)

            first_tile = [0] * NGR      # first global tile index per group
            acc = 0
            for gi, gsz in enumerate(GROUPS):
                first_tile[gi] = acc
                acc += gsz

            s_tiles = {}
            g_tiles = {}
            gp_tiles = {}

            def emit_conv_tile(gi, j):
                t = first_tile[gi] + j
                y0 = 4 * t
                cp = cps.tile([128, TPX], f32, tag="convps")
                ti = 0
                for dyp in range(3):
                    for kw in range(KS):
                        rhs = upad3[:, y0 + 2 * dyp : y0 + 2 * dyp + 4, kw : kw + W]
                        nc.tensor.matmul(
                            cp[:],
                            wt_t[:, ti * NOCH : (ti + 1) * NOCH],
                            rhs,
                            start=(ti == 0),
                            stop=(ti == NKT - 1),
                        )
                        ti += 1
                s_sl = s_tiles[gi][:, j * TPX : (j + 1) * TPX]
                nc.scalar.add(s_sl, cp[:], bias_t[:, 0:1])
                sq = sqpool.tile([128, TPX], f16, tag="sqt")
                nc.scalar.activation(sq[:], s_sl, AF.Square)
                return sq

            def emit_red(gi, j, sq):
                gsz = GROUPS[gi]
                nc.tensor.matmul(
                    gp_tiles[gi][:],
                    mred_t[:, j * 32 : j * 32 + 4 * gsz],
                    sq[:],
                    start=(j == 0),
                    stop=(j == gsz - 1),
                )

            def emit_chain(gi):
                gsz = GROUPS[gi]
                m = 4 * gsz
                gp = gp_tiles[gi]
                r4 = rr_t[0:m, gi * TPX : (gi + 1) * TPX]
                r1 = rr_t[0:m, (NGR + gi) * TPX : (NGR + gi + 1) * TPX]
                sqv = chpool.tile([32, TPX], f32, tag="sqv")
                nc.vector.tensor_mul(sqv[0:m, :], gp[:], r4)
                ln_a = chpool.tile([32, TPX], f32, tag="ln_a")
                nc.scalar.activation(
                    ln_a[0:m, :], sqv[0:m, :], AF.Ln, bias=eps_t[0:m, 0:1]
                )
                ln_b = chpool.tile([32, TPX], f32, tag="ln_b")
                nc.scalar.activation(ln_b[0:m, :], sqv[0:m, :], AF.Ln, bias=1.0)
                dd = chpool.tile([32, TPX], f32, tag="dd")
                nc.vector.scalar_tensor_tensor(
                    dd[0:m, :], ln_a[0:m, :], 0.5, ln_b[0:m, :], OP.mult, OP.subtract
                )
                ee = chpool.tile([32, TPX], f32, tag="ee")
                nc.scalar.activation(ee[0:m, :], dd[0:m, :], AF.Exp)
                gt_ = gvpool.tile([32, TPX], f16, tag="g32")
                nc.vector.tensor_mul(gt_[0:m, :], ee[0:m, :], r1)
                g_tiles[gi] = gt_

            def emit_phase2(gi, j):
                t = first_tile[gi] + j
                gsz = GROUPS[gi]
                gb = bps.tile([128, TPX], f32, tag="bcast")
                nc.tensor.matmul(
                    gb[:],
                    sel_t[0 : 4 * gsz, j * NOCH : (j + 1) * NOCH],
                    g_tiles[gi][0 : 4 * gsz, :],
                    start=True,
                    stop=True,
                )
                v = vpool.tile([128, TPX], f32, tag="vout")
                s_sl = s_tiles[gi][:, j * TPX : (j + 1) * TPX]
                nc.vector.tensor_mul(v[:], s_sl, gb[:])
                nc.sync.dma_start(out_d[:, t * TPX : (t + 1) * TPX], v[:])

            # Software-pipelined emission: the reduce for a tile is emitted one
            # conv-tile later (covers the ACT add+square latency), the chain as
            # soon as the group's last reduce is out, and phase2 work of group
            # g drains while group g+1's convs keep the PE busy.
            from collections import deque

            pend_red = deque()      # (gi, j, sq_tile)
            pend_p2 = deque()       # (gi, j)
            for gi, gsz in enumerate(GROUPS):
                s_tiles[gi] = sgpool.tile(
                    [128, gsz * TPX], f32, tag="sgroup", name=f"sgroup{gi}"
                )
                gp_tiles[gi] = gps.tile(
                    [4 * gsz, TPX], f32, tag="redps", name=f"redps{gi}"
                )
                for j in range(gsz):
                    emit_conv_tile_sq = emit_conv_tile(gi, j)
                    if pend_red:
                        rgi, rj, rsq = pend_red.popleft()
                        emit_red(rgi, rj, rsq)
                        if rj == GROUPS[rgi] - 1:
                            emit_chain(rgi)
                            pend_p2.extend((rgi, k) for k in range(GROUPS[rgi]))
                    pend_red.append((gi, j, emit_conv_tile_sq))
                    remaining = gsz - 1 - j
                    if pend_p2:
                        npop = max(1, -(-len(pend_p2) // max(1, remaining + 2)))
                        for _ in range(min(npop, len(pend_p2))):
                            emit_phase2(*pend_p2.popleft())
            # drain
            while pend_red:
                rgi, rj, rsq = pend_red.popleft()
                emit_red(rgi, rj, rsq)
                if rj == GROUPS[rgi] - 1:
                    emit_chain(rgi)
                    pend_p2.extend((rgi, k) for k in range(GROUPS[rgi]))
            while pend_p2:
                emit_phase2(*pend_p2.popleft())

    nc.compile()
    _BUILD_CACHE["nc"] = nc
    return nc


def _host_prep(u, Wf, bias):
    """Per-core input arrays. u [4,4,16,128,128], Wf [4,256,16,5,5], bias [4,256]."""
    u = np.ascontiguousarray(u, dtype=np.float32)
    Wf = np.ascontiguousarray(Wf, dtype=np.float32)
    bias = np.ascontiguousarray(bias, dtype=np.float32)

    # r(h,w) = 1/(8*nvalid); nvalid = clipped 5x5 window size
    nv = np.minimum(np.arange(H) + 2, H - 1) - np.maximum(np.arange(H) - 2, 0) + 1
    nvalid = np.outer(nv, nv).astype(np.float64)
    r = (1.0 / (8.0 * nvalid)).astype(np.float32)          # [H, W]

    # RR[p, gi*TPX + n] layout: row p -> (j = p//4), value depends on pixel only
    rr = np.zeros((32, 2 * NGR * TPX), np.float32)
    rflat = r.reshape(H * W)
    ft = 0
    for gi, gsz in enumerate(GROUPS):
        for p in range(4 * gsz):
            j = p // 4
            t = ft + j
            px = rflat[t * TPX : (t + 1) * TPX]
            rr[p, gi * TPX : (gi + 1) * TPX] = px * px
            rr[p, (NGR + gi) * TPX : (NGR + gi + 1) * TPX] = px
        ft += gsz

    # reduce masks: MRED[p, j*32+m] = 1 if m == 4*j + p//32
    mred = np.zeros((128, GT * 32), np.float16)
    for j in range(GT):
        for p in range(128):
            mred[p, j * 32 + 4 * j + p // 32] = 1.0
    # broadcast sel: SEL[p, j*128+m] = 1 if p == 4*j + m//32
    sel = np.zeros((32, GT * NOCH), np.float16)
    for j in range(GT):
        for m in range(NOCH):
            sel[4 * j + m // 32, j * NOCH + m] = 1.0

    bias_sum = bias.sum(axis=0)                            # [256]

    # weights: WT[p, ti*128 + o]; ti = dyp*5+kw; p = hlf64*64 + ic*16 + cid
    # lhsT[p, o] = W[ic, half*128+o, cid, 2*dyp+hlf64, kw] (0 for dy==5)
    wts = []
    for half in range(2):
        wt = np.zeros((128, NKT * NOCH), np.float16)
        Wh = Wf[:, half * NOCH : (half + 1) * NOCH]        # [4, 128, 16, 5, 5]
        for dyp in range(3):
            for kw in range(KS):
                ti = dyp * 5 + kw
                for h64 in range(2):
                    dy = 2 * dyp + h64
                    if dy >= KS:
                        continue
                    # [4,128,16] -> [4,16,128] -> [64,128]
                    blk = Wh[:, :, :, dy, kw].transpose(0, 2, 1).reshape(64, NOCH)
                    wt[h64 * 64 : (h64 + 1) * 64, ti * NOCH : (ti + 1) * NOCH] = blk
        wts.append(wt)

    # padded image per batch: pad[c, yy, xx]; second copy shifted +1 row
    upads = []
    for b in range(B):
        pad = np.zeros((CC, PADH, PADW), np.float16)
        pad[:, 2 : 2 + H, 2 : 2 + W] = u[b].reshape(CC, H, W)
        up = np.empty((128, PADH * PADW), np.float16)
        up[0:64] = pad.reshape(CC, -1)
        sh = np.zeros_like(pad)
        sh[:, 0 : PADH - 1] = pad[:, 1:PADH]
        up[64:128] = sh.reshape(CC, -1)
        upads.append(up)

    in_maps = []
    for c in range(8):
        b, half = c // 2, c % 2
        in_maps.append(
            {
                "upad": upads[b],
                "wt": wts[half],
                "bias": bias_sum[half * NOCH : (half + 1) * NOCH]
                .reshape(128, 1)
                .copy(),
                "mred": mred,
                "sel": sel,
                "rr": rr,
            }
        )
    return in_maps


def _gather(results):
    out = np.empty((B, OC, OD, H, W), np.float32)
    for c in range(8):
        b, half = c // 2, c % 2
        o = results[c]["out"]                              # [128, NPIX]
        out[b, half * 4 : (half + 1) * 4] = o.reshape(4, OD, H, W)
    return out


def run(u, W, bias, trace=False):
    _ensure_path()
    from concourse.bass_utils import run_bass_kernel_spmd

    nc = _build_program()
    in_maps = _host_prep(u, W, bias)
    res = run_bass_kernel_spmd(nc, in_maps, list(range(8)), trace=trace)
    return _gather(res.results), res


def kernel(u, W, bias):
    out, _ = run(u, W, bias, trace=False)
    return out


# revision 19
# speedup vs baseline: 1.1816x; 1.0158x over previous
"""CapsuleLayer kernel for 8x TRN2 NeuronCores (Bass/Tile, SPMD).

Math (reference collapses because routing logits b stay zero):
  s[b,o,h,w]  = sum_ic conv2d(u[b,ic], W[ic], SAME) + sum_ic bias[ic]
              = conv2d(u[b] as 64ch, Wcat[256,64,5,5]) + bias_sum      (one conv)
  r(h,w)      = 1 / (8 * nvalid(h,w))              (input-independent constant)
  p           = r * s ; sq[oc] = sum_od p^2
  v           = p * sq/((1+sq)*sqrt(sq+1e-9))
  out[b,oc,od,h,w] = v

Sharding: 8 cores = (batch b in 0..4) x (OC half in 0..2). Each core runs a
64->128-channel 5x5 conv over one 128x128 image + squash, fully on-chip.

Conv as matmuls: 15 accumulating PE matmuls per 512-pixel tile with K=128 =
2 kernel-rows x 64 in-channels (SBUF holds the padded image twice, the second
copy shifted one row, so paired kernel rows share one matmul). float32r
dtype: full PE speed (1 cyc/col at N=512) at ~1e-4 accuracy.

Squash: per-pixel cross-partition reduce/broadcast via PE matmuls with 0/1
masks (accumulating masked reduces build a [32,512] per-group tensor), the
scalar chain uses only {square, ln, exp} (one ACT table set):
  G = r * exp(0.5*ln(sq+1e-9) - ln(1+sq));  v = s * G_broadcast
"""

import numpy as np


def _ensure_path():
    try:
        import concourse.bass  # noqa: F401
    except ImportError:
        import sys

        for p in ("/opt/trn_rl_repo", "/root/.axon_site/_ro/trn_rl_repo"):
            if p not in sys.path:
                sys.path.insert(0, p)
        import concourse.bass  # noqa: F401


B, IC, CIN, H, W = 4, 4, 16, 128, 128
KS, OC, OD = 5, 8, 32
CC = IC * CIN            # 64 contraction channels
NOCH = 128               # out channels per core (4 capsules x 32 dims)
PADH, PADW = H + 5, W + 4   # 133 x 132 (extra pad row for the shifted copy)
NPIX = H * W
TPX = 512                # pixels per tile (4 rows)
NT = NPIX // TPX         # 32 tiles
GT = 8                   # max tiles per group (mask/sel layouts sized for this)
GROUPS = [8, 8, 8, 4, 4]  # tiles per squash group (small tail groups)
NGR = len(GROUPS)
NKT = 15                 # matmuls per conv tile: 3 row-pairs x 5 kw

_BUILD_CACHE = {}


def _build_program():
    """Build the SPMD Bass program (same for every core)."""
    if "nc" in _BUILD_CACHE:
        return _BUILD_CACHE["nc"]
    _ensure_path()
    import concourse.bacc as bacc
    import concourse.mybir as mybir
    import concourse.tile as tile

    f32 = mybir.dt.float32
    f16 = mybir.dt.float16
    AF = mybir.ActivationFunctionType
    OP = mybir.AluOpType

    # Square/Ln/Exp/Identity all live in the 'natural_log_exp_and_others' ACT
    # table set, but the default set picker uses a different home set per
    # function (2 table reloads x 1.3us per group). Restrict the choice so a
    # single table load covers the whole kernel.
    if not getattr(bacc, "_capsule_act_patch", False):
        _orig_tables = bacc.get_activation_tables

        def _one_set_tables(arch):
            t = _orig_tables(arch)
            keep = "natural_log_exp_and_others"
            if keep in t:
                t = {k: (v if k == keep else set()) for k, v in t.items()}
            return t

        bacc.get_activation_tables = _one_set_tables
        bacc._capsule_act_patch = True

    nc = bacc.Bacc("TRN2", target_bir_lowering=False, debug=False, num_devices=8)

    upad_d = nc.dram_tensor("upad", [128, PADH * PADW], f16, kind="ExternalInput").ap()
    wt_d = nc.dram_tensor("wt", [128, NKT * NOCH], f16, kind="ExternalInput").ap()
    bias_d = nc.dram_tensor("bias", [128, 1], f32, kind="ExternalInput").ap()
    mred_d = nc.dram_tensor("mred", [128, GT * 32], f16, kind="ExternalInput").ap()
    sel_d = nc.dram_tensor("sel", [32, GT * NOCH], f16, kind="ExternalInput").ap()
    rr_d = nc.dram_tensor("rr", [32, 2 * NGR * TPX], f32, kind="ExternalInput").ap()
    out_d = nc.dram_tensor("out", [128, NPIX], f32, kind="ExternalOutput").ap()

    with tile.TileContext(nc) as tc:
        with (
            tc.tile_pool(name="const", bufs=1) as cpool,
            tc.tile_pool(name="sg", bufs=3) as sgpool,
            tc.tile_pool(name="sq", bufs=3) as sqpool,
            tc.tile_pool(name="chain", bufs=2) as chpool,
            tc.tile_pool(name="gv", bufs=3) as gvpool,
            tc.tile_pool(name="vout", bufs=3) as vpool,
            tc.tile_pool(name="cps", bufs=4, space="PSUM") as cps,
            tc.tile_pool(name="gps", bufs=2, space="PSUM") as gps,
            tc.tile_pool(name="bps", bufs=2, space="PSUM") as bps,
        ):
            # DMA order matters: the first conv tiles need wt + the first image
            # rows. wt goes on the Sync HWDGE ring, the image chunks on the
            # Scalar HWDGE ring so the two streams land in parallel.
            wt_t = cpool.tile([128, NKT * NOCH], f16)
            nc.sync.dma_start(wt_t[:], wt_d[:])
            upad_t = cpool.tile([128, PADH * PADW], f16)
            upad3 = upad_t[:].rearrange("p (y x) -> p y x", x=PADW)
            usrc3 = upad_d.rearrange("p (y x) -> p y x", x=PADW)
            row_chunks = [(0, 14), (14, 44), (44, 74), (74, 104), (104, PADH)]
            for r0, r1 in row_chunks:
                nc.scalar.dma_start(upad3[:, r0:r1, :], usrc3[:, r0:r1, :])
            bias_t = cpool.tile([128, 1], f32)
            nc.sync.dma_start(bias_t[:], bias_d[:])
            mred_t = cpool.tile([128, GT * 32], f16)
            nc.sync.dma_start(mred_t[:], mred_d[:])
            sel_t = cpool.tile([32, GT * NOCH], f16)
            nc.sync.dma_start(sel_t[:], sel_d[:])
            rr_t = cpool.tile([32, 2 * NGR * TPX], f32)
            nc.sync.dma_start(rr_t[:], rr_d[:])
            eps_t = cpool.tile([128, 1], f32)
            nc.vector.memset(eps_t[:], 1e-9)

            first_tile = [0] * NGR      # first global tile index per group
            acc = 0
            for gi, gsz in enumerate(GROUPS):
                first_tile[gi] = acc
                acc += gsz

            s_tiles = {}
            g_tiles = {}
            gp_tiles = {}

            def emit_conv_tile(gi, j):
                t = first_tile[gi] + j
                y0 = 4 * t
                cp = cps.tile([128, TPX], f32, tag="convps")
                ti = 0
                for dyp in range(3):
                    for kw in range(KS):
                        rhs = upad3[:, y0 + 2 * dyp : y0 + 2 * dyp + 4, kw : kw + W]
                        nc.tensor.matmul(
                            cp[:],
                            wt_t[:, ti * NOCH : (ti + 1) * NOCH],
                            rhs,
                            start=(ti == 0),
                            stop=(ti == NKT - 1),
                        )
                        ti += 1
                # Square(cp + bias) straight from PSUM (fused bias, faster
                # PSUM read, and independent of the add below)
                sq = sqpool.tile([128, TPX], f16, tag="sqt")
                nc.scalar.activation(sq[:], cp[:], AF.Square, bias=bias_t[:, 0:1])
                s_sl = s_tiles[gi][:, j * TPX : (j + 1) * TPX]
                nc.scalar.add(s_sl, cp[:], bias_t[:, 0:1])
                return sq

            def emit_red(gi, j, sq):
                gsz = GROUPS[gi]
                nc.tensor.matmul(
                    gp_tiles[gi][:],
                    mred_t[:, j * 32 : j * 32 + 4 * gsz],
                    sq[:],
                    start=(j == 0),
                    stop=(j == gsz - 1),
                )

            def emit_chain(gi):
                gsz = GROUPS[gi]
                m = 4 * gsz
                gp = gp_tiles[gi]
                r4 = rr_t[0:m, gi * TPX : (gi + 1) * TPX]
                r1 = rr_t[0:m, (NGR + gi) * TPX : (NGR + gi + 1) * TPX]
                sqv = chpool.tile([32, TPX], f32, tag="sqv")
                nc.vector.tensor_mul(sqv[0:m, :], gp[:], r4)
                ln_a = chpool.tile([32, TPX], f32, tag="ln_a")
                nc.scalar.activation(
                    ln_a[0:m, :], sqv[0:m, :], AF.Ln, bias=eps_t[0:m, 0:1]
                )
                ln_b = chpool.tile([32, TPX], f32, tag="ln_b")
                nc.scalar.activation(ln_b[0:m, :], sqv[0:m, :], AF.Ln, bias=1.0)
                dd = chpool.tile([32, TPX], f32, tag="dd")
                nc.vector.scalar_tensor_tensor(
                    dd[0:m, :], ln_a[0:m, :], 0.5, ln_b[0:m, :], OP.mult, OP.subtract
                )
                ee = chpool.tile([32, TPX], f32, tag="ee")
                nc.scalar.activation(ee[0:m, :], dd[0:m, :], AF.Exp)
                gt_ = gvpool.tile([32, TPX], f16, tag="g32")
                nc.vector.tensor_mul(gt_[0:m, :], ee[0:m, :], r1)
                g_tiles[gi] = gt_

            def emit_phase2(gi, j):
                t = first_tile[gi] + j
                gsz = GROUPS[gi]
                gb = bps.tile([128, TPX], f32, tag="bcast")
                nc.tensor.matmul(
                    gb[:],
                    sel_t[0 : 4 * gsz, j * NOCH : (j + 1) * NOCH],
                    g_tiles[gi][0 : 4 * gsz, :],
                    start=True,
                    stop=True,
                )
                v = vpool.tile([128, TPX], f32, tag="vout")
                s_sl = s_tiles[gi][:, j * TPX : (j + 1) * TPX]
                nc.vector.tensor_mul(v[:], s_sl, gb[:])
                nc.sync.dma_start(out_d[:, t * TPX : (t + 1) * TPX], v[:])

            # Software-pipelined emission: the reduce for a tile is emitted one
            # conv-tile later (covers the ACT add+square latency), the chain as
            # soon as the group's last reduce is out, and phase2 work of group
            # g drains while group g+1's convs keep the PE busy.
            from collections import deque

            pend_red = deque()      # (gi, j, sq_tile)
            pend_p2 = deque()       # (gi, j)
            tiles_left = NT
            for gi, gsz in enumerate(GROUPS):
                s_tiles[gi] = sgpool.tile(
                    [128, gsz * TPX], f32, tag="sgroup", name=f"sgroup{gi}"
                )
                gp_tiles[gi] = gps.tile(
                    [4 * gsz, TPX], f32, tag="redps", name=f"redps{gi}"
                )
                for j in range(gsz):
                    emit_conv_tile_sq = emit_conv_tile(gi, j)
                    tiles_left -= 1
                    if pend_red:
                        rgi, rj, rsq = pend_red.popleft()
                        emit_red(rgi, rj, rsq)
                        if rj == GROUPS[rgi] - 1:
                            emit_chain(rgi)
                            pend_p2.extend((rgi, k) for k in range(GROUPS[rgi]))
                    pend_red.append((gi, j, emit_conv_tile_sq))
                    # hold back a few phase2 items so PE has fill work for the
                    # final chain's latency at the kernel tail
                    excess = len(pend_p2) - 5
                    if excess > 0:
                        npop = -(-excess // max(1, tiles_left))
                        for _ in range(min(npop, len(pend_p2))):
                            emit_phase2(*pend_p2.popleft())
            # drain
            while pend_red:
                rgi, rj, rsq = pend_red.popleft()
                emit_red(rgi, rj, rsq)
                if rj == GROUPS[rgi] - 1:
                    emit_chain(rgi)
                    pend_p2.extend((rgi, k) for k in range(GROUPS[rgi]))
            while pend_p2:
                emit_phase2(*pend_p2.popleft())

    nc.compile()
    _BUILD_CACHE["nc"] = nc
    return nc


def _host_prep(u, Wf, bias):
    """Per-core input arrays. u [4,4,16,128,128], Wf [4,256,16,5,5], bias [4,256]."""
    u = np.ascontiguousarray(u, dtype=np.float32)
    Wf = np.ascontiguousarray(Wf, dtype=np.float32)
    bias = np.ascontiguousarray(bias, dtype=np.float32)

    # r(h,w) = 1/(8*nvalid); nvalid = clipped 5x5 window size
    nv = np.minimum(np.arange(H) + 2, H - 1) - np.maximum(np.arange(H) - 2, 0) + 1
    nvalid = np.outer(nv, nv).astype(np.float64)
    r = (1.0 / (8.0 * nvalid)).astype(np.float32)          # [H, W]

    # RR[p, gi*TPX + n] layout: row p -> (j = p//4), value depends on pixel only
    rr = np.zeros((32, 2 * NGR * TPX), np.float32)
    rflat = r.reshape(H * W)
    ft = 0
    for gi, gsz in enumerate(GROUPS):
        for p in range(4 * gsz):
            j = p // 4
            t = ft + j
            px = rflat[t * TPX : (t + 1) * TPX]
            rr[p, gi * TPX : (gi + 1) * TPX] = px * px
            rr[p, (NGR + gi) * TPX : (NGR + gi + 1) * TPX] = px
        ft += gsz

    # reduce masks: MRED[p, j*32+m] = 1 if m == 4*j + p//32
    mred = np.zeros((128, GT * 32), np.float16)
    for j in range(GT):
        for p in range(128):
            mred[p, j * 32 + 4 * j + p // 32] = 1.0
    # broadcast sel: SEL[p, j*128+m] = 1 if p == 4*j + m//32
    sel = np.zeros((32, GT * NOCH), np.float16)
    for j in range(GT):
        for m in range(NOCH):
            sel[4 * j + m // 32, j * NOCH + m] = 1.0

    bias_sum = bias.sum(axis=0)                            # [256]

    # weights: WT[p, ti*128 + o]; ti = dyp*5+kw; p = hlf64*64 + ic*16 + cid
    # lhsT[p, o] = W[ic, half*128+o, cid, 2*dyp+hlf64, kw] (0 for dy==5)
    wts = []
    for half in range(2):
        wt = np.zeros((128, NKT * NOCH), np.float16)
        Wh = Wf[:, half * NOCH : (half + 1) * NOCH]        # [4, 128, 16, 5, 5]
        for dyp in range(3):
            for kw in range(KS):
                ti = dyp * 5 + kw
                for h64 in range(2):
                    dy = 2 * dyp + h64
                    if dy >= KS:
                        continue
                    # [4,128,16] -> [4,16,128] -> [64,128]
                    blk = Wh[:, :, :, dy, kw].transpose(0, 2, 1).reshape(64, NOCH)
                    wt[h64 * 64 : (h64 + 1) * 64, ti * NOCH : (ti + 1) * NOCH] = blk
        wts.append(wt)

    # padded image per batch: pad[c, yy, xx]; second copy shifted +1 row
    upads = []
    for b in range(B):
        pad = np.zeros((CC, PADH, PADW), np.float16)
        pad[:, 2 : 2 + H, 2 : 2 + W] = u[b].reshape(CC, H, W)
        up = np.empty((128, PADH * PADW), np.float16)
        up[0:64] = pad.reshape(CC, -1)
        sh = np.zeros_like(pad)
        sh[:, 0 : PADH - 1] = pad[:, 1:PADH]
        up[64:128] = sh.reshape(CC, -1)
        upads.append(up)

    in_maps = []
    for c in range(8):
        b, half = c // 2, c % 2
        in_maps.append(
            {
                "upad": upads[b],
                "wt": wts[half],
                "bias": bias_sum[half * NOCH : (half + 1) * NOCH]
                .reshape(128, 1)
                .copy(),
                "mred": mred,
                "sel": sel,
                "rr": rr,
            }
        )
    return in_maps


def _gather(results):
    out = np.empty((B, OC, OD, H, W), np.float32)
    for c in range(8):
        b, half = c // 2, c % 2
        o = results[c]["out"]                              # [128, NPIX]
        out[b, half * 4 : (half + 1) * 4] = o.reshape(4, OD, H, W)
    return out


def run(u, W, bias, trace=False):
    _ensure_path()
    from concourse.bass_utils import run_bass_kernel_spmd

    nc = _build_program()
    in_maps = _host_prep(u, W, bias)
    res = run_bass_kernel_spmd(nc, in_maps, list(range(8)), trace=trace)
    return _gather(res.results), res


def kernel(u, W, bias):
    out, _ = run(u, W, bias, trace=False)
    return out
